# revision 1
# baseline (speedup 1.0000x reference)
import sys

sys.path.insert(0, "/opt/trn_rl_repo")

import numpy as np

# Runtime knobs (test.py may set these before calling kernel()).
TRACE = False
USE_SIM = False
LAST_EXEC_NS = None
LAST_PROFILE = None

P = 128          # SBUF partitions
DIM = 32
NCORES = 8
N_NODES = 65536
NPC = N_NODES // NCORES   # nodes per core = 8192
S = NPC // P              # slots per partition = 64
ITERS = 8
CHUNK_MAX = 96            # max gather-chunk width in slots

_CACHE = {}


def _relu(a):
    return np.maximum(a, 0.0)


def _preprocess(inputs):
    x = np.asarray(inputs["x"], dtype=np.float32)
    ei = np.asarray(inputs["edge_index"]).astype(np.int64)
    ea = np.asarray(inputs["edge_attr"], dtype=np.float32).reshape(-1)
    lin0_w = np.asarray(inputs["lin0_w"], np.float32)
    lin0_b = np.asarray(inputs["lin0_b"], np.float32)
    nn_w1 = np.asarray(inputs["nn_w1"], np.float32)
    nn_b1 = np.asarray(inputs["nn_b1"], np.float32)
    nn_w2 = np.asarray(inputs["nn_w2"], np.float32)
    nn_b2 = np.asarray(inputs["nn_b2"], np.float32)
    root = np.asarray(inputs["root"], np.float32)
    conv_bias = np.asarray(inputs["conv_bias"], np.float32)

    N = x.shape[0]
    assert N == N_NODES and ei.shape[1] == 4 * N

    # W[e] = ea_e * B requires relu(ea*w1 + b1) == ea * relu(w1): b1 == 0, ea >= 0.
    assert np.all(nn_b1 == 0.0) and np.all(nn_b2 == 0.0) and float(ea.min()) >= 0.0, (
        "kernel specialization requires nn_b1 == nn_b2 == 0 and edge_attr >= 0"
    )

    h0 = _relu(x @ lin0_w + lin0_b).astype(np.float32)                # [N, 32]
    Bmat = (_relu(nn_w1) @ nn_w2).reshape(DIM, DIM).astype(np.float32)
    broot = np.ascontiguousarray(
        np.concatenate([Bmat, root], axis=1), dtype=np.float32
    )                                                                  # [32, 64]

    src, dst = ei[0], ei[1]
    owner = dst // NPC

    grow = np.empty(N, np.int64)      # node -> global HB-table row
    percore = []
    for c in range(NCORES):
        m = owner == c
        d_local = dst[m] - c * NPC
        s_c = src[m]
        e_c = ea[m]
        deg = np.bincount(d_local, minlength=NPC)
        order = np.argsort(-deg, kind="stable")   # rank -> local node
        rk = np.empty(NPC, np.int64)
        rk[order] = np.arange(NPC)                # local node -> rank
        degs = deg[order]                         # descending
        grow[c * NPC:(c + 1) * NPC] = c * NPC + (rk % P) * S + rk // P
        percore.append((d_local, s_c, e_c, order, rk, degs))

    Rmax = int(max(pc[5][0] for pc in percore))
    k_r = np.zeros(Rmax, np.int64)
    for pc in percore:
        degs = pc[5]
        a = -degs  # ascending
        cr = np.searchsorted(a, -np.arange(Rmax), side="left")  # count(degs > r)
        k_r = np.maximum(k_r, (cr + P - 1) // P)
    k_r[0] = S
    off = np.zeros(Rmax + 1, np.int64)
    off[1:] = np.cumsum(k_r)
    K_tot = int(off[-1])

    # groups of rounds >= 1 packed into gather chunks <= CHUNK_MAX slots wide
    groups = []  # [col_offset, width, [(local_off, kr), ...]]
    for r in range(1, Rmax):
        kr = int(k_r[r])
        if groups and groups[-1][1] + kr <= CHUNK_MAX:
            g = groups[-1]
            g[2].append((g[1], kr))
            g[1] += kr
        else:
            groups.append([int(off[r]), kr, [(0, kr)]])
    # (col_offset, width, round list, is_round0)
    chunks = [(0, S, [(0, S)], True)] + [(g[0], g[1], g[2], False) for g in groups]

    per_core_arrays = []
    for c, (d_local, s_c, e_c, order, rk, degs) in enumerate(percore):
        r_e = rk[d_local]
        o = np.argsort(r_e, kind="stable")
        rs = r_e[o]
        ss = s_c[o]
        es = e_c[o]
        startpos = np.searchsorted(rs, np.arange(NPC), side="left")
        j = np.arange(len(rs)) - startpos[rs]     # occurrence index = round
        col = off[j] + rs // P
        p = rs % P
        idx_a = np.zeros((P, K_tot), np.int32)
        ea_a = np.zeros((P, K_tot, 1), np.float32)
        idx_a[p, col] = grow[ss].astype(np.int32)
        ea_a[p, col, 0] = es
        inv_a = np.zeros((P, S, 1), np.float32)
        u = np.arange(NPC)
        inv_full = np.where(degs > 0, 1.0 / np.maximum(degs, 1), 0.0).astype(np.float32)
        inv_a[u % P, u // P, 0] = inv_full
        h0_sel = h0[c * NPC + order]              # [8192, 32] in rank order
        h0_a = np.ascontiguousarray(
            h0_sel.reshape(S, P, DIM).transpose(1, 0, 2)
        )                                          # [128, 64, 32]; h0_a[p,s] = rank s*P+p
        per_core_arrays.append(dict(h0=h0_a, idx=idx_a, ea=ea_a, inv=inv_a,
                                    broot=broot))

    bias_nonzero = bool(np.any(conv_bias != 0.0))
    if bias_nonzero:
        bias_a = np.ascontiguousarray(
            np.broadcast_to(conv_bias.reshape(1, 1, DIM), (P, 1, DIM)),
            dtype=np.float32)
        for d in per_core_arrays:
            d["cbias"] = bias_a

    orders = [pc[3] for pc in percore]
    meta = dict(K_tot=K_tot, chunks=chunks, bias_nonzero=bias_nonzero,
                orders=orders,
                lin2_w=np.asarray(inputs["lin2_w"], np.float32),
                lin2_b=np.asarray(inputs["lin2_b"], np.float32))
    return per_core_arrays, meta


def _build_program(K_tot, chunks, bias_nonzero):
    from concourse import bacc, bass, mybir, tile
    from concourse.masks import make_identity

    f32 = mybir.dt.float32
    bf16 = mybir.dt.bfloat16
    i32 = mybir.dt.int32
    MULT = mybir.AluOpType.mult

    nc = bacc.Bacc("TRN2", target_bir_lowering=False, debug=False,
                   num_devices=NCORES)

    h0_p = nc.dram_tensor("h0", [P, S, DIM], f32, kind="ExternalInput").ap()
    idx_p = nc.dram_tensor("idx", [P, K_tot], i32, kind="ExternalInput").ap()
    ea_p = nc.dram_tensor("ea", [P, K_tot, 1], f32, kind="ExternalInput").ap()
    inv_p = nc.dram_tensor("inv", [P, S, 1], f32, kind="ExternalInput").ap()
    br_p = nc.dram_tensor("broot", [DIM, 2 * DIM], f32, kind="ExternalInput").ap()
    if bias_nonzero:
        cb_p = nc.dram_tensor("cbias", [P, 1, DIM], f32, kind="ExternalInput").ap()
    hout_p = nc.dram_tensor("h_out", [P, S, DIM], f32, kind="ExternalOutput").ap()

    with tile.TileContext(nc) as tc:
        with (
            tc.tile_pool(name="persist", bufs=1) as pp,
            tc.tile_pool(name="work", bufs=2) as wp,
            tc.tile_pool(name="gpool", bufs=1) as gp,
            tc.tile_pool(name="dramp", bufs=2, space="DRAM") as dp,
            tc.tile_pool(name="pst", bufs=2, space="PSUM") as pst,
            tc.tile_pool(name="psm", bufs=2, space="PSUM") as psm,
        ):
            ident = pp.tile([P, P], f32)
            make_identity(nc, ident[:])
            h = pp.tile([P, S, DIM], f32)
            idx_sb = pp.tile([P, K_tot], i32)
            ea_sb = pp.tile([P, K_tot, 1], f32)
            inv_sb = pp.tile([P, S, 1], f32)
            br_sb = pp.tile([DIM, 2 * DIM], f32)
            nc.sync.dma_start(out=h[:], in_=h0_p[:])
            nc.sync.dma_start(out=idx_sb[:], in_=idx_p[:])
            nc.sync.dma_start(out=ea_sb[:], in_=ea_p[:])
            nc.sync.dma_start(out=inv_sb[:], in_=inv_p[:])
            nc.sync.dma_start(out=br_sb[:], in_=br_p[:])
            if bias_nonzero:
                cb_sb = pp.tile([P, 1, DIM], f32)
                nc.sync.dma_start(out=cb_sb[:], in_=cb_p[:])

            for it in range(ITERS):
                # hT[:, c, :] = h[:, c, :]^T  (feature-major copy for matmul lhsT)
                hT = wp.tile([DIM, S, P], f32)
                for tb in range(S // 4):
                    pt = pst.tile([DIM, 4, P], f32)
                    for b in range(4):
                        nc.tensor.transpose(out=pt[:, b, :], in_=h[:, tb * 4 + b, :],
                                            identity=ident[:])
                    nc.any.tensor_copy(out=hT[:, tb * 4:tb * 4 + 4, :], in_=pt[:])

                # [HB | hR] = h @ [B | root], node-major
                # HB is AllGathered + gathered in bf16 to halve DMA bytes
                hbc = wp.tile([P, S, DIM], bf16)
                hr = wp.tile([P, S, DIM], f32)
                for mb in range(S // 8):
                    pm = psm.tile([P, 8, 2 * DIM], f32)
                    for b in range(8):
                        cidx = mb * 8 + b
                        nc.tensor.matmul(out=pm[:, b, :], lhsT=hT[:, cidx, :],
                                         rhs=br_sb[:], start=True, stop=True)
                    nc.any.tensor_copy(out=hbc[:, mb * 8:mb * 8 + 8, :],
                                       in_=pm[:, :, 0:DIM])
                    nc.any.tensor_copy(out=hr[:, mb * 8:mb * 8 + 8, :],
                                       in_=pm[:, :, DIM:2 * DIM])

                bounce = dp.tile([NPC, DIM], bf16)
                hbf = dp.tile([N_NODES, DIM], bf16)
                nc.sync.dma_start(
                    out=bounce[:].rearrange("(p s) d -> p s d", p=P), in_=hbc[:])
                nc.gpsimd.collective_compute(
                    "AllGather", mybir.AluOpType.bypass,
                    replica_groups=[list(range(NCORES))],
                    ins=[bounce.opt()], outs=[hbf.opt()],
                )

                # gather + scale + segment-sum (ELL rounds)
                agg = wp.tile([P, S, DIM], f32)
                for (coff, width, rlist, is_first) in chunks:
                    g = gp.tile([P, width, DIM], bf16, name=f"g{coff}")
                    # HW indirect DMA only supports one offset per partition
                    for j in range(width):
                        nc.gpsimd.indirect_dma_start(
                            out=g[:, j, :], out_offset=None, in_=hbf[:],
                            in_offset=bass.IndirectOffsetOnAxis(
                                ap=idx_sb[:, coff + j:coff + j + 1], axis=0),
                        )
                    gf = gp.tile([P, width, DIM], f32, name=f"gf{coff}")
                    nc.any.tensor_copy(out=gf[:], in_=g[:])
                    eab = ea_sb[:, coff:coff + width, :].to_broadcast([P, width, DIM])
                    if is_first:
                        nc.vector.tensor_tensor(out=agg[:], in0=gf[:], in1=eab,
                                                op=MULT)
                    else:
                        nc.vector.tensor_tensor(out=gf[:], in0=gf[:], in1=eab,
                                                op=MULT)
                        for (lo, kr) in rlist:
                            nc.vector.tensor_add(out=agg[:, 0:kr, :],
                                                 in0=agg[:, 0:kr, :],
                                                 in1=gf[:, lo:lo + kr, :])

                # h += relu(agg * inv + h @ root (+ bias))
                nc.vector.tensor_tensor(out=agg[:], in0=agg[:],
                                        in1=inv_sb[:].to_broadcast([P, S, DIM]),
                                        op=MULT)
                nc.vector.tensor_add(out=agg[:], in0=agg[:], in1=hr[:])
                if bias_nonzero:
                    nc.vector.tensor_add(out=agg[:], in0=agg[:],
                                         in1=cb_sb[:].to_broadcast([P, S, DIM]))
                nc.scalar.activation(out=agg[:], in_=agg[:],
                                     func=mybir.ActivationFunctionType.Relu)
                nc.vector.tensor_add(out=h[:], in0=h[:], in1=agg[:])

            nc.sync.dma_start(out=hout_p[:], in_=h[:])

    nc.compile()
    return nc


TIME_K = 9        # chained executions in the timing jit
TIME_REPS = 5     # wall-clock repetitions, take min
_RUNNERS = {}


def _pjrt_runner(nc):
    import jax
    from jax.experimental.shard_map import shard_map
    from jax.sharding import Mesh, NamedSharding, PartitionSpec
    from concourse import mybir
    from concourse.bass2jax import (_bass_exec_p, install_neuronx_cc_hook,
                                    partition_id_tensor)

    install_neuronx_cc_hook()

    partition_name = nc.partition_id_tensor.name if nc.partition_id_tensor else None
    in_names, out_names, out_avals = [], [], []
    for alloc in nc.m.functions[0].allocations:
        if not isinstance(alloc, mybir.MemoryLocationSet):
            continue
        name = alloc.memorylocations[0].name
        if alloc.kind == "ExternalInput":
            if name != partition_name:
                in_names.append(name)
        elif alloc.kind == "ExternalOutput":
            out_names.append(name)
            out_avals.append(jax.core.ShapedArray(
                tuple(alloc.tensor_shape), mybir.dt.np(alloc.dtype)))
    n_params = len(in_names)
    all_names = tuple(in_names) + tuple(out_names) + (
        (partition_name,) if partition_name else ())

    def bind(ins, carries):
        ops = list(ins) + list(carries)
        if partition_name is not None:
            ops.append(partition_id_tensor())
        return _bass_exec_p.bind(
            *ops, out_avals=tuple(out_avals), in_names=all_names,
            out_names=tuple(out_names), lowering_input_output_aliases=(),
            sim_require_finite=True, sim_require_nnan=True, nc=nc)

    def body1(*args):
        return tuple(bind(args[:n_params], args[n_params:]))

    devices = jax.devices()[:NCORES]
    mesh = Mesh(np.asarray(devices), ("core",))
    spec = PartitionSpec("core")
    nio = n_params + len(out_names)
    f1 = jax.jit(shard_map(body1, mesh=mesh, in_specs=(spec,) * nio,
                           out_specs=(spec,) * len(out_names), check_rep=False))
    sharding = NamedSharding(mesh, spec)
    return dict(in_names=in_names, out_names=out_names, out_avals=out_avals,
                sharding=sharding, f1=f1, jax=jax)


def _pjrt_run_maps(nc, in_maps, time_it=False):
    global LAST_EXEC_NS, LAST_PROFILE
    import time as _time
    r = _RUNNERS.get(id(nc))
    if r is None:
        r = _pjrt_runner(nc)
        _RUNNERS[id(nc)] = r
    jax = r["jax"]
    concat_in = [np.concatenate([in_maps[c][nm] for c in range(NCORES)], axis=0)
                 for nm in r["in_names"]]
    zeros = [np.zeros((NCORES * a.shape[0], *a.shape[1:]), a.dtype)
             for a in r["out_avals"]]
    dev_in = [jax.device_put(x, r["sharding"]) for x in concat_in]
    dev_zero = [jax.device_put(z, r["sharding"]) for z in zeros]

    outs = jax.block_until_ready(r["f1"](*dev_in, *dev_zero))

    if time_it:
        # One bass_exec per jit module is allowed, so chain K executions by
        # issuing K async dispatches back-to-back; they queue on-device and
        # the slope vs a single blocked call removes the host/tunnel RTT.
        t1 = tk = float("inf")
        for _ in range(TIME_REPS):
            t0 = _time.perf_counter()
            jax.block_until_ready(r["f1"](*dev_in, *dev_zero))
            t1 = min(t1, _time.perf_counter() - t0)
            t0 = _time.perf_counter()
            rs = [r["f1"](*dev_in, *dev_zero) for _ in range(TIME_K)]
            jax.block_until_ready(rs)
            tk = min(tk, _time.perf_counter() - t0)
        LAST_EXEC_NS = int((tk - t1) / (TIME_K - 1) * 1e9)
        LAST_PROFILE = {"t1_ns": int(t1 * 1e9), "tK_ns": int(tk * 1e9),
                        "K": TIME_K}

    out_full = np.asarray(outs[0]).reshape(NCORES, *r["out_avals"][0].shape)
    return [out_full[c] for c in range(NCORES)]


def _run(nc, per_core_arrays):
    in_maps = [dict(d) for d in per_core_arrays]

    if USE_SIM:
        from concourse.bass_interp import MultiCoreSim
        sim = MultiCoreSim(nc, num_cores=NCORES)
        for i in range(NCORES):
            for k, v in in_maps[i].items():
                sim.cores[i].tensor(k)[:] = v
        sim.simulate()
        return [np.array(sim.cores[i].tensor("h_out")) for i in range(NCORES)]

    return _pjrt_run_maps(nc, in_maps, time_it=TRACE)


def kernel(**inputs):
    per_core_arrays, meta = _preprocess(inputs)

    key = (meta["K_tot"], tuple((c[0], c[1]) for c in meta["chunks"]),
           meta["bias_nonzero"])
    nc = _CACHE.get(key)
    if nc is None:
        nc = _build_program(meta["K_tot"], meta["chunks"], meta["bias_nonzero"])
        _CACHE[key] = nc

    outs = _run(nc, per_core_arrays)

    h_full = np.empty((N_NODES, DIM), np.float32)
    for c in range(NCORES):
        by_rank = np.asarray(outs[c]).reshape(P, S, DIM).transpose(1, 0, 2).reshape(NPC, DIM)
        h_full[c * NPC + meta["orders"][c]] = by_rank
    g = np.mean(h_full, axis=0, dtype=np.float64).astype(np.float32)
    out = (g @ meta["lin2_w"] + meta["lin2_b"]).reshape(-1).astype(np.float32)
    return out



# revision 5
# speedup vs baseline: 4.5119x; 4.5119x over previous
import sys

sys.path.insert(0, "/opt/trn_rl_repo")

import numpy as np

# Runtime knobs (test.py may set these before calling kernel()).
TRACE = False
USE_SIM = False
LAST_EXEC_NS = None
LAST_PROFILE = None

P = 128          # SBUF partitions
DIM = 32
NCORES = 8
N_NODES = 65536
NPC = N_NODES // NCORES   # nodes per core = 8192
S = NPC // P              # slots per partition = 64
ITERS = 8
CHUNK_MAX = 64            # max DVE-chunk width in ELL columns
GIDX = 1024               # indices per dma_gather (HW ucode limit)
GCOLS = GIDX // P         # ELL columns per gather = 8
NQUEUES = 4

_CACHE = {}


def _relu(a):
    return np.maximum(a, 0.0)


def _preprocess(inputs):
    x = np.asarray(inputs["x"], dtype=np.float32)
    ei = np.asarray(inputs["edge_index"]).astype(np.int64)
    ea = np.asarray(inputs["edge_attr"], dtype=np.float32).reshape(-1)
    lin0_w = np.asarray(inputs["lin0_w"], np.float32)
    lin0_b = np.asarray(inputs["lin0_b"], np.float32)
    nn_w1 = np.asarray(inputs["nn_w1"], np.float32)
    nn_b1 = np.asarray(inputs["nn_b1"], np.float32)
    nn_w2 = np.asarray(inputs["nn_w2"], np.float32)
    nn_b2 = np.asarray(inputs["nn_b2"], np.float32)
    root = np.asarray(inputs["root"], np.float32)
    conv_bias = np.asarray(inputs["conv_bias"], np.float32)

    N = x.shape[0]
    assert N == N_NODES and ei.shape[1] == 4 * N

    # W[e] = ea_e * B requires relu(ea*w1 + b1) == ea * relu(w1): b1 == 0, ea >= 0.
    assert np.all(nn_b1 == 0.0) and np.all(nn_b2 == 0.0) and float(ea.min()) >= 0.0, (
        "kernel specialization requires nn_b1 == nn_b2 == 0 and edge_attr >= 0"
    )

    h0 = _relu(x @ lin0_w + lin0_b).astype(np.float32)                # [N, 32]
    Bmat = (_relu(nn_w1) @ nn_w2).reshape(DIM, DIM).astype(np.float32)
    import ml_dtypes
    broot = np.ascontiguousarray(
        np.concatenate([Bmat, root], axis=1)).astype(ml_dtypes.bfloat16)  # [32, 64]

    src, dst = ei[0], ei[1]
    owner = dst // NPC

    grow = np.empty(N, np.int64)      # node -> global HB-table row
    percore = []
    for c in range(NCORES):
        m = owner == c
        d_local = dst[m] - c * NPC
        s_c = src[m]
        e_c = ea[m]
        deg = np.bincount(d_local, minlength=NPC)
        order = np.argsort(-deg, kind="stable")   # rank -> local node
        rk = np.empty(NPC, np.int64)
        rk[order] = np.arange(NPC)                # local node -> rank
        degs = deg[order]                         # descending
        grow[c * NPC:(c + 1) * NPC] = c * NPC + (rk % P) * S + rk // P
        percore.append((d_local, s_c, e_c, order, rk, degs))

    Rmax = int(max(pc[5][0] for pc in percore))
    k_r = np.zeros(Rmax, np.int64)
    for pc in percore:
        degs = pc[5]
        a = -degs  # ascending
        cr = np.searchsorted(a, -np.arange(Rmax), side="left")  # count(degs > r)
        k_r = np.maximum(k_r, (cr + P - 1) // P)
    k_r[0] = S
    off = np.zeros(Rmax + 1, np.int64)
    off[1:] = np.cumsum(k_r)
    K_tot = int(off[-1])
    K4 = -(-K_tot // GCOLS) * GCOLS           # pad to a whole number of gathers
    NG = K4 // GCOLS

    # DVE chunks: round 0, then groups of rounds >= 1 packed <= CHUNK_MAX wide
    groups = []  # [col_offset, width, [(local_off, kr), ...]]
    for r in range(1, Rmax):
        kr = int(k_r[r])
        if groups and groups[-1][1] + kr <= CHUNK_MAX:
            g = groups[-1]
            g[2].append((g[1], kr))
            g[1] += kr
        else:
            groups.append([int(off[r]), kr, [(0, kr)]])
    chunks = [(0, S, [(0, S)], True)] + [(g[0], g[1], g[2], False) for g in groups]

    per_core_arrays = []
    for c, (d_local, s_c, e_c, order, rk, degs) in enumerate(percore):
        r_e = rk[d_local]
        o = np.argsort(r_e, kind="stable")
        rs = r_e[o]
        ss = s_c[o]
        es = e_c[o]
        startpos = np.searchsorted(rs, np.arange(NPC), side="left")
        j = np.arange(len(rs)) - startpos[rs]     # occurrence index = round
        col = off[j] + rs // P
        p = rs % P
        sidx = np.zeros((P, K4), np.int64)        # global table row of src
        ea4 = np.zeros((P, K4, 4), np.float32)
        sidx[p, col] = grow[ss]
        ea4[p, col, grow[ss] % 4] = es
        sup = (sidx // 4).astype(np.int16)        # super-row (4 nodes / 256 B)

        # dma_gather index layout: gather i covers columns [Gi, Gi+GCOLS);
        # slot j = (col-Gi)*P + p; idx wrapped into 16 partitions, x8 replicas.
        blocks = []
        for i in range(NG):
            flat = sup[:, i * GCOLS:(i + 1) * GCOLS].T.reshape(GIDX)
            blocks.append(flat.reshape(GIDX // 16, 16).T)
        idxg = np.concatenate(blocks, axis=1)     # [16, NG*64]
        idxg = np.tile(idxg, (8, 1)).copy()       # [128, NG*64]

        inv_a = np.zeros((P, S, 1), np.float32)
        u = np.arange(NPC)
        inv_full = np.where(degs > 0, 1.0 / np.maximum(degs, 1), 0.0).astype(np.float32)
        inv_a[u % P, u // P, 0] = inv_full
        h0_sel = h0[c * NPC + order]              # [8192, 32] in rank order
        h0_a = np.ascontiguousarray(
            h0_sel.reshape(S, P, DIM).transpose(1, 0, 2)
        )                                          # [128, 64, 32]; h0_a[p,s] = rank s*P+p
        per_core_arrays.append(dict(h0=h0_a, idxg=idxg, ea4=ea4, inv=inv_a,
                                    broot=broot))

    bias_nonzero = bool(np.any(conv_bias != 0.0))
    if bias_nonzero:
        bias_a = np.ascontiguousarray(
            np.broadcast_to(conv_bias.reshape(1, 1, DIM), (P, 1, DIM)),
            dtype=np.float32)
        for d in per_core_arrays:
            d["cbias"] = bias_a

    orders = [pc[3] for pc in percore]
    meta = dict(K_tot=K_tot, K4=K4, NG=NG, chunks=chunks,
                bias_nonzero=bias_nonzero, orders=orders,
                lin2_w=np.asarray(inputs["lin2_w"], np.float32),
                lin2_b=np.asarray(inputs["lin2_b"], np.float32))
    return per_core_arrays, meta


def _build_program(K_tot, K4, NG, chunks, bias_nonzero):
    from concourse import bacc, bass, mybir, tile
    from concourse import library_config
    from concourse.masks import make_identity

    f32 = mybir.dt.float32
    bf16 = mybir.dt.bfloat16
    i16 = mybir.dt.int16
    MULT = mybir.AluOpType.mult

    nc = bacc.Bacc("TRN2", target_bir_lowering=False, debug=False,
                   num_devices=NCORES, num_swdge_queues=NQUEUES)

    h0_p = nc.dram_tensor("h0", [P, S, DIM], f32, kind="ExternalInput").ap()
    idxg_p = nc.dram_tensor("idxg", [P, NG * (GIDX // 16)], i16,
                            kind="ExternalInput").ap()
    ea4_p = nc.dram_tensor("ea4", [P, K4, 4], f32, kind="ExternalInput").ap()
    inv_p = nc.dram_tensor("inv", [P, S, 1], f32, kind="ExternalInput").ap()
    br_p = nc.dram_tensor("broot", [DIM, 2 * DIM], bf16, kind="ExternalInput").ap()
    if bias_nonzero:
        cb_p = nc.dram_tensor("cbias", [P, 1, DIM], f32, kind="ExternalInput").ap()
    hout_p = nc.dram_tensor("h_out", [P, S, DIM], f32, kind="ExternalOutput").ap()

    with tile.TileContext(nc) as tc:
        with (
            tc.tile_pool(name="persist", bufs=1) as pp,
            tc.tile_pool(name="work", bufs=2) as wp,
            tc.tile_pool(name="gpool", bufs=1) as gp,
            tc.tile_pool(name="prodp", bufs=2) as prp,
            tc.tile_pool(name="dramp", bufs=2, space="DRAM") as dp,
            tc.tile_pool(name="pst", bufs=2, space="PSUM") as pst,
            tc.tile_pool(name="psm", bufs=2, space="PSUM") as psm,
        ):
            nc.gpsimd.load_library(library_config.mlp)
            ident = pp.tile([P, P], f32)
            make_identity(nc, ident[:])
            h = pp.tile([P, S, DIM], f32)
            idxg_sb = pp.tile([P, NG * (GIDX // 16)], i16)
            ea4_sb = pp.tile([P, K4, 4], f32)
            inv_sb = pp.tile([P, S, 1], f32)
            br_sb = pp.tile([DIM, 2 * DIM], bf16)
            gbuf = pp.tile([P, K4, 4 * DIM], bf16)
            nc.sync.dma_start(out=h[:], in_=h0_p[:])
            nc.sync.dma_start(out=idxg_sb[:], in_=idxg_p[:])
            nc.sync.dma_start(out=ea4_sb[:], in_=ea4_p[:])
            nc.sync.dma_start(out=inv_sb[:], in_=inv_p[:])
            nc.sync.dma_start(out=br_sb[:], in_=br_p[:])
            if bias_nonzero:
                cb_sb = pp.tile([P, 1, DIM], f32)
                nc.sync.dma_start(out=cb_sb[:], in_=cb_p[:])

            for it in range(ITERS):
                # hT[:, c, :] = h[:, c, :]^T  (feature-major copy for matmul lhsT)
                hT = wp.tile([DIM, S, P], bf16)
                for tb in range(S // 4):
                    pt = pst.tile([DIM, 4, P], f32)
                    for b in range(4):
                        nc.tensor.transpose(out=pt[:, b, :], in_=h[:, tb * 4 + b, :],
                                            identity=ident[:])
                    nc.any.tensor_copy(out=hT[:, tb * 4:tb * 4 + 4, :], in_=pt[:])

                # [HB | hR] = h @ [B | root], node-major
                # HB is AllGathered + gathered in bf16 to halve DMA bytes
                hbc = wp.tile([P, S, DIM], bf16)
                hr = wp.tile([P, S, DIM], f32)
                for mb in range(S // 8):
                    pm = psm.tile([P, 8, 2 * DIM], f32)
                    for b in range(8):
                        cidx = mb * 8 + b
                        nc.tensor.matmul(out=pm[:, b, :], lhsT=hT[:, cidx, :],
                                         rhs=br_sb[:], start=True, stop=True)
                    nc.any.tensor_copy(out=hbc[:, mb * 8:mb * 8 + 8, :],
                                       in_=pm[:, :, 0:DIM])
                    nc.any.tensor_copy(out=hr[:, mb * 8:mb * 8 + 8, :],
                                       in_=pm[:, :, DIM:2 * DIM])

                bounce = dp.tile([NPC, DIM], bf16)
                hbf = dp.tile([N_NODES, DIM], bf16)
                nc.sync.dma_start(
                    out=bounce[:].rearrange("(p s) d -> p s d", p=P), in_=hbc[:])
                nc.gpsimd.collective_compute(
                    "AllGather", mybir.AluOpType.bypass,
                    replica_groups=[list(range(NCORES))],
                    ins=[bounce.opt()], outs=[hbf.opt()],
                )
                hbf4 = hbf[:].rearrange("(s f) d -> s (f d)", f=4)  # [16384, 128]

                # batched gathers of 256-B super-rows (4 nodes each).
                # queue = (sem % 8) % NQUEUES keeps the tile DMASW sem
                # rotation (mod 8) consistently paired with SWDGE queues.
                for i in range(NG):
                    g_ctr = it * NG + i
                    nc.gpsimd.dma_gather(
                        gbuf[:, i * GCOLS:(i + 1) * GCOLS, :], hbf4,
                        idxg_sb[:, i * (GIDX // 16):(i + 1) * (GIDX // 16)],
                        GIDX, GIDX, 4 * DIM,
                        queue_num=(g_ctr % 8) % NQUEUES)

                # masked-ea multiply (4 lanes) + fold + segment-sum (ELL rounds)
                agg = wp.tile([P, S, DIM], f32)
                for (coff, width, rlist, is_first) in chunks:
                    tdst = agg[:, 0:width, :] if is_first else None
                    t = prp.tile([P, CHUNK_MAX, DIM], f32, name="t")
                    s = prp.tile([P, CHUNK_MAX, DIM], f32, name="s")
                    tv = tdst if is_first else t[:, 0:width, :]
                    for m in range(4):
                        gsl = gbuf[:, coff:coff + width, m * DIM:(m + 1) * DIM]
                        eas = ea4_sb[:, coff:coff + width, m:m + 1].to_broadcast(
                            [P, width, DIM])
                        if m == 0:
                            nc.vector.tensor_tensor(out=tv, in0=gsl, in1=eas,
                                                    op=MULT)
                        else:
                            nc.vector.tensor_tensor(out=s[:, 0:width, :], in0=gsl,
                                                    in1=eas, op=MULT)
                            nc.vector.tensor_add(out=tv, in0=tv,
                                                 in1=s[:, 0:width, :])
                    if not is_first:
                        for (lo, kr) in rlist:
                            nc.vector.tensor_add(out=agg[:, 0:kr, :],
                                                 in0=agg[:, 0:kr, :],
                                                 in1=t[:, lo:lo + kr, :])

                # h += relu(agg * inv + h @ root (+ bias))
                nc.vector.tensor_tensor(out=agg[:], in0=agg[:],
                                        in1=inv_sb[:].to_broadcast([P, S, DIM]),
                                        op=MULT)
                nc.vector.tensor_add(out=agg[:], in0=agg[:], in1=hr[:])
                if bias_nonzero:
                    nc.vector.tensor_add(out=agg[:], in0=agg[:],
                                         in1=cb_sb[:].to_broadcast([P, S, DIM]))
                nc.scalar.activation(out=agg[:], in_=agg[:],
                                     func=mybir.ActivationFunctionType.Relu)
                nc.vector.tensor_add(out=h[:], in0=h[:], in1=agg[:])

            nc.sync.dma_start(out=hout_p[:], in_=h[:])

    nc.compile()
    return nc


TIME_K = 9        # chained executions in the timing jit
TIME_REPS = 5     # wall-clock repetitions, take min
_RUNNERS = {}


def _pjrt_runner(nc):
    import jax
    from jax.experimental.shard_map import shard_map
    from jax.sharding import Mesh, NamedSharding, PartitionSpec
    from concourse import mybir
    from concourse.bass2jax import (_bass_exec_p, install_neuronx_cc_hook,
                                    partition_id_tensor)

    install_neuronx_cc_hook()

    partition_name = nc.partition_id_tensor.name if nc.partition_id_tensor else None
    in_names, out_names, out_avals = [], [], []
    for alloc in nc.m.functions[0].allocations:
        if not isinstance(alloc, mybir.MemoryLocationSet):
            continue
        name = alloc.memorylocations[0].name
        if alloc.kind == "ExternalInput":
            if name != partition_name:
                in_names.append(name)
        elif alloc.kind == "ExternalOutput":
            out_names.append(name)
            out_avals.append(jax.core.ShapedArray(
                tuple(alloc.tensor_shape), mybir.dt.np(alloc.dtype)))
    n_params = len(in_names)
    all_names = tuple(in_names) + tuple(out_names) + (
        (partition_name,) if partition_name else ())

    def bind(ins, carries):
        ops = list(ins) + list(carries)
        if partition_name is not None:
            ops.append(partition_id_tensor())
        return _bass_exec_p.bind(
            *ops, out_avals=tuple(out_avals), in_names=all_names,
            out_names=tuple(out_names), lowering_input_output_aliases=(),
            sim_require_finite=True, sim_require_nnan=True, nc=nc)

    def body1(*args):
        return tuple(bind(args[:n_params], args[n_params:]))

    devices = jax.devices()[:NCORES]
    mesh = Mesh(np.asarray(devices), ("core",))
    spec = PartitionSpec("core")
    nio = n_params + len(out_names)
    f1 = jax.jit(shard_map(body1, mesh=mesh, in_specs=(spec,) * nio,
                           out_specs=(spec,) * len(out_names), check_rep=False))
    sharding = NamedSharding(mesh, spec)
    return dict(in_names=in_names, out_names=out_names, out_avals=out_avals,
                sharding=sharding, f1=f1, jax=jax)


def _pjrt_run_maps(nc, in_maps, time_it=False):
    global LAST_EXEC_NS, LAST_PROFILE
    import time as _time
    r = _RUNNERS.get(id(nc))
    if r is None:
        r = _pjrt_runner(nc)
        _RUNNERS[id(nc)] = r
    jax = r["jax"]
    concat_in = [np.concatenate([in_maps[c][nm] for c in range(NCORES)], axis=0)
                 for nm in r["in_names"]]
    zeros = [np.zeros((NCORES * a.shape[0], *a.shape[1:]), a.dtype)
             for a in r["out_avals"]]
    dev_in = [jax.device_put(x, r["sharding"]) for x in concat_in]
    dev_zero = [jax.device_put(z, r["sharding"]) for z in zeros]

    outs = jax.block_until_ready(r["f1"](*dev_in, *dev_zero))

    if time_it:
        # One bass_exec per jit module is allowed, so chain K executions by
        # issuing K async dispatches back-to-back; they queue on-device and
        # the slope vs a single blocked call removes the host/tunnel RTT.
        t1 = tk = float("inf")
        for _ in range(TIME_REPS):
            t0 = _time.perf_counter()
            jax.block_until_ready(r["f1"](*dev_in, *dev_zero))
            t1 = min(t1, _time.perf_counter() - t0)
            t0 = _time.perf_counter()
            rs = [r["f1"](*dev_in, *dev_zero) for _ in range(TIME_K)]
            jax.block_until_ready(rs)
            tk = min(tk, _time.perf_counter() - t0)
        LAST_EXEC_NS = int((tk - t1) / (TIME_K - 1) * 1e9)
        LAST_PROFILE = {"t1_ns": int(t1 * 1e9), "tK_ns": int(tk * 1e9),
                        "K": TIME_K}

    out_full = np.asarray(outs[0]).reshape(NCORES, *r["out_avals"][0].shape)
    return [out_full[c] for c in range(NCORES)]


def _run(nc, per_core_arrays):
    in_maps = [dict(d) for d in per_core_arrays]

    if USE_SIM:
        from concourse.bass_interp import MultiCoreSim
        sim = MultiCoreSim(nc, num_cores=NCORES)
        for i in range(NCORES):
            for k, v in in_maps[i].items():
                sim.cores[i].tensor(k)[:] = v
        sim.simulate()
        return [np.array(sim.cores[i].tensor("h_out")) for i in range(NCORES)]

    return _pjrt_run_maps(nc, in_maps, time_it=TRACE)


def kernel(**inputs):
    per_core_arrays, meta = _preprocess(inputs)

    key = (meta["K_tot"], meta["K4"],
           tuple((c[0], c[1]) for c in meta["chunks"]),
           meta["bias_nonzero"])
    nc = _CACHE.get(key)
    if nc is None:
        nc = _build_program(meta["K_tot"], meta["K4"], meta["NG"],
                            meta["chunks"], meta["bias_nonzero"])
        _CACHE[key] = nc

    outs = _run(nc, per_core_arrays)

    h_full = np.empty((N_NODES, DIM), np.float32)
    for c in range(NCORES):
        by_rank = np.asarray(outs[c]).reshape(P, S, DIM).transpose(1, 0, 2).reshape(NPC, DIM)
        h_full[c * NPC + meta["orders"][c]] = by_rank
    g = np.mean(h_full, axis=0, dtype=np.float64).astype(np.float32)
    out = (g @ meta["lin2_w"] + meta["lin2_b"]).reshape(-1).astype(np.float32)
    return out


# revision 7
# speedup vs baseline: 5.3682x; 1.1898x over previous
import sys

sys.path.insert(0, "/opt/trn_rl_repo")

import numpy as np
import ml_dtypes

# Runtime knobs (test.py may set these before calling kernel()).
TRACE = False
USE_SIM = False
LAST_EXEC_NS = None
LAST_PROFILE = None

P = 128          # SBUF partitions
DIM = 32
NCORES = 8
N_NODES = 65536
NPC = N_NODES // NCORES   # nodes per core = 8192
S = NPC // P              # slots per partition = 64
ITERS = 8
CHUNK_MAX = 64            # max DVE-chunk width in ELL columns
GIDX = 1024               # indices per dma_gather (HW ucode limit)
GCOLS = GIDX // P         # ELL columns per gather = 8
NQUEUES = 4

_CACHE = {}


def _relu(a):
    return np.maximum(a, 0.0)


def _preprocess(inputs):
    x = np.asarray(inputs["x"], dtype=np.float32)
    ei = np.asarray(inputs["edge_index"]).astype(np.int64)
    ea = np.asarray(inputs["edge_attr"], dtype=np.float32).reshape(-1)
    lin0_w = np.asarray(inputs["lin0_w"], np.float32)
    lin0_b = np.asarray(inputs["lin0_b"], np.float32)
    nn_w1 = np.asarray(inputs["nn_w1"], np.float32)
    nn_b1 = np.asarray(inputs["nn_b1"], np.float32)
    nn_w2 = np.asarray(inputs["nn_w2"], np.float32)
    nn_b2 = np.asarray(inputs["nn_b2"], np.float32)
    root = np.asarray(inputs["root"], np.float32)
    conv_bias = np.asarray(inputs["conv_bias"], np.float32)

    N = x.shape[0]
    assert N == N_NODES and ei.shape[1] == 4 * N

    # W[e] = ea_e * B requires relu(ea*w1 + b1) == ea * relu(w1): b1 == 0, ea >= 0.
    assert np.all(nn_b1 == 0.0) and np.all(nn_b2 == 0.0) and float(ea.min()) >= 0.0, (
        "kernel specialization requires nn_b1 == nn_b2 == 0 and edge_attr >= 0"
    )

    h0 = _relu(x @ lin0_w + lin0_b).astype(np.float32)                # [N, 32]
    Bmat = (_relu(nn_w1) @ nn_w2).reshape(DIM, DIM).astype(np.float32)
    broot = np.ascontiguousarray(
        np.concatenate([Bmat, root], axis=1)).astype(ml_dtypes.bfloat16)  # [32, 64]

    src, dst = ei[0], ei[1]
    owner = dst // NPC

    grow = np.empty(N, np.int64)      # node -> global HB-table row
    percore = []
    for c in range(NCORES):
        m = owner == c
        d_local = dst[m] - c * NPC
        s_c = src[m]
        e_c = ea[m]
        deg = np.bincount(d_local, minlength=NPC)
        order = np.argsort(-deg, kind="stable")   # rank -> local node
        rk = np.empty(NPC, np.int64)
        rk[order] = np.arange(NPC)                # local node -> rank
        degs = deg[order]                         # descending
        grow[c * NPC:(c + 1) * NPC] = c * NPC + (rk % P) * S + rk // P
        percore.append((d_local, s_c, e_c, order, rk, degs))

    Rmax = int(max(pc[5][0] for pc in percore))
    k_r = np.zeros(Rmax, np.int64)
    for pc in percore:
        degs = pc[5]
        a = -degs  # ascending
        cr = np.searchsorted(a, -np.arange(Rmax), side="left")  # count(degs > r)
        k_r = np.maximum(k_r, (cr + P - 1) // P)
    k_r[0] = S
    off = np.zeros(Rmax + 1, np.int64)
    off[1:] = np.cumsum(k_r)
    K_tot = int(off[-1])
    K4 = -(-K_tot // GCOLS) * GCOLS           # pad to a whole number of gathers
    NG = K4 // GCOLS

    # DVE chunks: round 0, then groups of rounds >= 1 packed <= CHUNK_MAX wide
    groups = []  # [col_offset, width, [(local_off, kr), ...]]
    for r in range(1, Rmax):
        kr = int(k_r[r])
        if groups and groups[-1][1] + kr <= CHUNK_MAX:
            g = groups[-1]
            g[2].append((g[1], kr))
            g[1] += kr
        else:
            groups.append([int(off[r]), kr, [(0, kr)]])
    chunks = [(0, S, [(0, S)], True)] + [(g[0], g[1], g[2], False) for g in groups]

    per_core_arrays = []
    for c, (d_local, s_c, e_c, order, rk, degs) in enumerate(percore):
        r_e = rk[d_local]
        o = np.argsort(r_e, kind="stable")
        rs = r_e[o]
        ss = s_c[o]
        es = e_c[o]
        startpos = np.searchsorted(rs, np.arange(NPC), side="left")
        j = np.arange(len(rs)) - startpos[rs]     # occurrence index = round
        col = off[j] + rs // P
        p = rs % P
        inv_full = np.where(degs > 0, 1.0 / np.maximum(degs, 1), 0.0).astype(
            np.float32)                           # rank -> 1/deg
        sidx = np.zeros((P, K4), np.int64)        # global table row of src
        ea4 = np.zeros((P, K4, 4), np.float32)
        sidx[p, col] = grow[ss]
        ea4[p, col, grow[ss] % 4] = es * inv_full[rs]   # mean-fold: ea/deg(dst)
        sup = (sidx // 4).astype(np.int16)        # super-row (4 nodes / 256 B)

        # dma_gather index layout: gather i covers columns [Gi, Gi+GCOLS);
        # slot j = (col-Gi)*P + p; idx wrapped into 16 partitions, x8 replicas.
        blocks = []
        for i in range(NG):
            flat = sup[:, i * GCOLS:(i + 1) * GCOLS].T.reshape(GIDX)
            blocks.append(flat.reshape(GIDX // 16, 16).T)
        idxg = np.concatenate(blocks, axis=1)     # [16, NG*64]
        idxg = np.tile(idxg, (8, 1)).copy()       # [128, NG*64]

        h0_sel = h0[c * NPC + order]              # [8192, 32] in rank order
        h0_a = np.ascontiguousarray(
            h0_sel.reshape(S, P, DIM).transpose(1, 0, 2)
        )                                          # [128, 64, 32]; h0_a[p,s] = rank s*P+p
        ea4 = ea4.astype(ml_dtypes.bfloat16)
        per_core_arrays.append(dict(h0=h0_a, idxg=idxg, ea4=ea4,
                                    broot=broot))

    bias_nonzero = bool(np.any(conv_bias != 0.0))
    if bias_nonzero:
        bias_a = np.ascontiguousarray(
            np.broadcast_to(conv_bias.reshape(1, 1, DIM), (P, 1, DIM)),
            dtype=np.float32)
        for d in per_core_arrays:
            d["cbias"] = bias_a

    orders = [pc[3] for pc in percore]
    meta = dict(K_tot=K_tot, K4=K4, NG=NG, chunks=chunks,
                bias_nonzero=bias_nonzero, orders=orders,
                lin2_w=np.asarray(inputs["lin2_w"], np.float32),
                lin2_b=np.asarray(inputs["lin2_b"], np.float32))
    return per_core_arrays, meta


def _build_program(K_tot, K4, NG, chunks, bias_nonzero, iters=ITERS):
    from concourse import bacc, bass, mybir, tile
    from concourse import library_config
    from concourse.masks import make_identity

    f32 = mybir.dt.float32
    bf16 = mybir.dt.bfloat16
    i16 = mybir.dt.int16
    MULT = mybir.AluOpType.mult

    nc = bacc.Bacc("TRN2", target_bir_lowering=False, debug=False,
                   num_devices=NCORES, num_swdge_queues=NQUEUES)

    h0_p = nc.dram_tensor("h0", [P, S, DIM], f32, kind="ExternalInput").ap()
    idxg_p = nc.dram_tensor("idxg", [P, NG * (GIDX // 16)], i16,
                            kind="ExternalInput").ap()
    ea4_p = nc.dram_tensor("ea4", [P, K4, 4], bf16, kind="ExternalInput").ap()
    br_p = nc.dram_tensor("broot", [DIM, 2 * DIM], bf16, kind="ExternalInput").ap()
    if bias_nonzero:
        cb_p = nc.dram_tensor("cbias", [P, 1, DIM], f32, kind="ExternalInput").ap()
    hout_p = nc.dram_tensor("h_out", [P, S, DIM], f32, kind="ExternalOutput").ap()

    with tile.TileContext(nc) as tc:
        with (
            tc.tile_pool(name="persist", bufs=1) as pp,
            tc.tile_pool(name="work", bufs=2) as wp,
            tc.tile_pool(name="gpool", bufs=1) as gp,
            tc.tile_pool(name="prodp", bufs=2) as prp,
            tc.tile_pool(name="dramp", bufs=2, space="DRAM") as dp,
            tc.tile_pool(name="pst", bufs=2, space="PSUM") as pst,
            tc.tile_pool(name="psm", bufs=2, space="PSUM") as psm,
        ):
            nc.gpsimd.load_library(library_config.mlp)
            ident = pp.tile([P, P], f32)
            make_identity(nc, ident[:])
            h = pp.tile([P, S, DIM], f32)
            idxg_sb = pp.tile([P, NG * (GIDX // 16)], i16)
            ea4_sb = pp.tile([P, K4, 4], bf16)
            br_sb = pp.tile([DIM, 2 * DIM], bf16)
            gbuf = pp.tile([P, K4, 4 * DIM], bf16)
            nc.sync.dma_start(out=h[:], in_=h0_p[:])
            nc.sync.dma_start(out=idxg_sb[:], in_=idxg_p[:])
            nc.sync.dma_start(out=ea4_sb[:], in_=ea4_p[:])
            nc.sync.dma_start(out=br_sb[:], in_=br_p[:])
            if bias_nonzero:
                cb_sb = pp.tile([P, 1, DIM], f32)
                nc.sync.dma_start(out=cb_sb[:], in_=cb_p[:])

            for it in range(iters):
                # hT[:, c, :] = h[:, c, :]^T  (feature-major copy for matmul lhsT)
                hT = wp.tile([DIM, S, P], bf16)
                for tb in range(S // 4):
                    pt = pst.tile([DIM, 4, P], f32)
                    for b in range(4):
                        nc.tensor.transpose(out=pt[:, b, :], in_=h[:, tb * 4 + b, :],
                                            identity=ident[:])
                    nc.any.tensor_copy(out=hT[:, tb * 4:tb * 4 + 4, :], in_=pt[:])

                # [HB | hR] = h @ [B | root], node-major
                # HB is AllGathered + gathered in bf16 to halve DMA bytes
                hbc = wp.tile([P, S, DIM], bf16)
                hr = wp.tile([P, S, DIM], f32)
                for mb in range(S // 8):
                    pm = psm.tile([P, 8, 2 * DIM], f32)
                    for b in range(8):
                        cidx = mb * 8 + b
                        nc.tensor.matmul(out=pm[:, b, :], lhsT=hT[:, cidx, :],
                                         rhs=br_sb[:], start=True, stop=True)
                    nc.any.tensor_copy(out=hbc[:, mb * 8:mb * 8 + 8, :],
                                       in_=pm[:, :, 0:DIM])
                    nc.any.tensor_copy(out=hr[:, mb * 8:mb * 8 + 8, :],
                                       in_=pm[:, :, DIM:2 * DIM])

                bounce = dp.tile([NPC, DIM], bf16)
                hbf = dp.tile([N_NODES, DIM], bf16, addr_space="Shared")
                nc.sync.dma_start(
                    out=bounce[:].rearrange("(p s) d -> p s d", p=P), in_=hbc[:])
                nc.gpsimd.collective_compute(
                    "AllGather", mybir.AluOpType.bypass,
                    replica_groups=[list(range(NCORES))],
                    ins=[bounce.opt()], outs=[hbf.opt()],
                )
                hbf4 = hbf[:].rearrange("(s f) d -> s (f d)", f=4)  # [16384, 128]

                # batched gathers of 256-B super-rows (4 nodes each).
                # queue = (sem % 8) % NQUEUES keeps the tile DMASW sem
                # rotation (mod 8) consistently paired with SWDGE queues.
                for i in range(NG):
                    g_ctr = it * NG + i
                    nc.gpsimd.dma_gather(
                        gbuf[:, i * GCOLS:(i + 1) * GCOLS, :], hbf4,
                        idxg_sb[:, i * (GIDX // 16):(i + 1) * (GIDX // 16)],
                        GIDX, GIDX, 4 * DIM,
                        queue_num=(g_ctr % 8) % NQUEUES)

                # masked-ea multiply (4 lanes) + fold + segment-sum (ELL rounds)
                agg = wp.tile([P, S, DIM], f32)
                for (coff, width, rlist, is_first) in chunks:
                    tdst = agg[:, 0:width, :] if is_first else None
                    t = prp.tile([P, CHUNK_MAX, DIM], bf16, name="t")
                    s = prp.tile([P, CHUNK_MAX, DIM], bf16, name="s")
                    tv = tdst if is_first else t[:, 0:width, :]
                    for m in range(4):
                        gsl = gbuf[:, coff:coff + width, m * DIM:(m + 1) * DIM]
                        eas = ea4_sb[:, coff:coff + width, m:m + 1].to_broadcast(
                            [P, width, DIM])
                        if m == 0:
                            nc.vector.tensor_tensor(out=tv, in0=gsl, in1=eas,
                                                    op=MULT)
                        else:
                            nc.vector.tensor_tensor(out=s[:, 0:width, :], in0=gsl,
                                                    in1=eas, op=MULT)
                            nc.vector.tensor_add(out=tv, in0=tv,
                                                 in1=s[:, 0:width, :])
                    if not is_first:
                        for (lo, kr) in rlist:
                            nc.vector.tensor_add(out=agg[:, 0:kr, :],
                                                 in0=agg[:, 0:kr, :],
                                                 in1=t[:, lo:lo + kr, :])

                # h += relu(agg + h @ root (+ bias)); 1/deg folded into ea4
                nc.vector.tensor_add(out=agg[:], in0=agg[:], in1=hr[:])
                if bias_nonzero:
                    nc.vector.tensor_add(out=agg[:], in0=agg[:],
                                         in1=cb_sb[:].to_broadcast([P, S, DIM]))
                nc.scalar.activation(out=agg[:], in_=agg[:],
                                     func=mybir.ActivationFunctionType.Relu)
                nc.vector.tensor_add(out=h[:], in0=h[:], in1=agg[:])

            nc.sync.dma_start(out=hout_p[:], in_=h[:])

    nc.compile()
    return nc


TIME_K = 9        # chained executions in the timing jit
TIME_REPS = 5     # wall-clock repetitions, take min
_RUNNERS = {}


def _pjrt_runner(nc):
    import jax
    from jax.experimental.shard_map import shard_map
    from jax.sharding import Mesh, NamedSharding, PartitionSpec
    from concourse import mybir
    from concourse.bass2jax import (_bass_exec_p, install_neuronx_cc_hook,
                                    partition_id_tensor)

    install_neuronx_cc_hook()

    partition_name = nc.partition_id_tensor.name if nc.partition_id_tensor else None
    in_names, out_names, out_avals = [], [], []
    for alloc in nc.m.functions[0].allocations:
        if not isinstance(alloc, mybir.MemoryLocationSet):
            continue
        name = alloc.memorylocations[0].name
        if alloc.kind == "ExternalInput":
            if name != partition_name:
                in_names.append(name)
        elif alloc.kind == "ExternalOutput":
            out_names.append(name)
            out_avals.append(jax.core.ShapedArray(
                tuple(alloc.tensor_shape), mybir.dt.np(alloc.dtype)))
    n_params = len(in_names)
    all_names = tuple(in_names) + tuple(out_names) + (
        (partition_name,) if partition_name else ())

    def bind(ins, carries):
        ops = list(ins) + list(carries)
        if partition_name is not None:
            ops.append(partition_id_tensor())
        return _bass_exec_p.bind(
            *ops, out_avals=tuple(out_avals), in_names=all_names,
            out_names=tuple(out_names), lowering_input_output_aliases=(),
            sim_require_finite=True, sim_require_nnan=True, nc=nc)

    def body1(*args):
        return tuple(bind(args[:n_params], args[n_params:]))

    devices = jax.devices()[:NCORES]
    mesh = Mesh(np.asarray(devices), ("core",))
    spec = PartitionSpec("core")
    nio = n_params + len(out_names)
    f1 = jax.jit(shard_map(body1, mesh=mesh, in_specs=(spec,) * nio,
                           out_specs=(spec,) * len(out_names), check_rep=False))
    sharding = NamedSharding(mesh, spec)
    return dict(in_names=in_names, out_names=out_names, out_avals=out_avals,
                sharding=sharding, f1=f1, jax=jax)


def _pjrt_run_maps(nc, in_maps, time_it=False):
    global LAST_EXEC_NS, LAST_PROFILE
    import time as _time
    r = _RUNNERS.get(id(nc))
    if r is None:
        r = _pjrt_runner(nc)
        _RUNNERS[id(nc)] = r
    jax = r["jax"]
    concat_in = [np.concatenate([in_maps[c][nm] for c in range(NCORES)], axis=0)
                 for nm in r["in_names"]]
    zeros = [np.zeros((NCORES * a.shape[0], *a.shape[1:]), a.dtype)
             for a in r["out_avals"]]
    dev_in = [jax.device_put(x, r["sharding"]) for x in concat_in]
    dev_zero = [jax.device_put(z, r["sharding"]) for z in zeros]

    outs = jax.block_until_ready(r["f1"](*dev_in, *dev_zero))

    if time_it:
        # One bass_exec per jit module is allowed, so chain K executions by
        # issuing K async dispatches back-to-back; they queue on-device and
        # the slope vs a single blocked call removes the host/tunnel RTT.
        t1 = tk = float("inf")
        for _ in range(TIME_REPS):
            t0 = _time.perf_counter()
            jax.block_until_ready(r["f1"](*dev_in, *dev_zero))
            t1 = min(t1, _time.perf_counter() - t0)
            t0 = _time.perf_counter()
            rs = [r["f1"](*dev_in, *dev_zero) for _ in range(TIME_K)]
            jax.block_until_ready(rs)
            tk = min(tk, _time.perf_counter() - t0)
        LAST_EXEC_NS = int((tk - t1) / (TIME_K - 1) * 1e9)
        LAST_PROFILE = {"t1_ns": int(t1 * 1e9), "tK_ns": int(tk * 1e9),
                        "K": TIME_K}

    out_full = np.asarray(outs[0]).reshape(NCORES, *r["out_avals"][0].shape)
    return [out_full[c] for c in range(NCORES)]


def _run(nc, per_core_arrays):
    in_maps = [dict(d) for d in per_core_arrays]

    if USE_SIM:
        from concourse.bass_interp import MultiCoreSim
        sim = MultiCoreSim(nc, num_cores=NCORES)
        for i in range(NCORES):
            for k, v in in_maps[i].items():
                sim.cores[i].tensor(k)[:] = v
        sim.simulate()
        return [np.array(sim.cores[i].tensor("h_out")) for i in range(NCORES)]

    return _pjrt_run_maps(nc, in_maps, time_it=TRACE)


def kernel(**inputs):
    per_core_arrays, meta = _preprocess(inputs)

    key = (meta["K_tot"], meta["K4"],
           tuple((c[0], c[1]) for c in meta["chunks"]),
           meta["bias_nonzero"])
    nc = _CACHE.get(key)
    if nc is None:
        nc = _build_program(meta["K_tot"], meta["K4"], meta["NG"],
                            meta["chunks"], meta["bias_nonzero"])
        _CACHE[key] = nc

    outs = _run(nc, per_core_arrays)

    h_full = np.empty((N_NODES, DIM), np.float32)
    for c in range(NCORES):
        by_rank = np.asarray(outs[c]).reshape(P, S, DIM).transpose(1, 0, 2).reshape(NPC, DIM)
        h_full[c * NPC + meta["orders"][c]] = by_rank
    g = np.mean(h_full, axis=0, dtype=np.float64).astype(np.float32)
    out = (g @ meta["lin2_w"] + meta["lin2_b"]).reshape(-1).astype(np.float32)
    return out


# revision 9
# speedup vs baseline: 5.9795x; 1.1139x over previous
import sys

sys.path.insert(0, "/opt/trn_rl_repo")

import numpy as np
import ml_dtypes

# Runtime knobs (test.py may set these before calling kernel()).
TRACE = False
USE_SIM = False
LAST_EXEC_NS = None
LAST_PROFILE = None

P = 128          # SBUF partitions
DIM = 32
NCORES = 8
N_NODES = 65536
NPC = N_NODES // NCORES   # nodes per core = 8192
S = NPC // P              # slots per partition = 64
ITERS = 8
CHUNK_MAX = 32            # max DVE-chunk width in ELL columns
GIDX = 1024               # indices per dma_gather (HW ucode limit)
GCOLS = GIDX // P         # ELL columns per gather = 8
NQUEUES = 4

_CACHE = {}


def _relu(a):
    return np.maximum(a, 0.0)


def _preprocess(inputs):
    x = np.asarray(inputs["x"], dtype=np.float32)
    ei = np.asarray(inputs["edge_index"]).astype(np.int64)
    ea = np.asarray(inputs["edge_attr"], dtype=np.float32).reshape(-1)
    lin0_w = np.asarray(inputs["lin0_w"], np.float32)
    lin0_b = np.asarray(inputs["lin0_b"], np.float32)
    nn_w1 = np.asarray(inputs["nn_w1"], np.float32)
    nn_b1 = np.asarray(inputs["nn_b1"], np.float32)
    nn_w2 = np.asarray(inputs["nn_w2"], np.float32)
    nn_b2 = np.asarray(inputs["nn_b2"], np.float32)
    root = np.asarray(inputs["root"], np.float32)
    conv_bias = np.asarray(inputs["conv_bias"], np.float32)

    N = x.shape[0]
    assert N == N_NODES and ei.shape[1] == 4 * N

    # W[e] = ea_e * B requires relu(ea*w1 + b1) == ea * relu(w1): b1 == 0, ea >= 0.
    assert np.all(nn_b1 == 0.0) and np.all(nn_b2 == 0.0) and float(ea.min()) >= 0.0, (
        "kernel specialization requires nn_b1 == nn_b2 == 0 and edge_attr >= 0"
    )

    h0 = _relu(x @ lin0_w + lin0_b).astype(np.float32)                # [N, 32]
    Bmat = (_relu(nn_w1) @ nn_w2).reshape(DIM, DIM).astype(np.float32)
    broot = np.ascontiguousarray(
        np.concatenate([Bmat, root], axis=1)).astype(ml_dtypes.bfloat16)  # [32, 64]

    src, dst = ei[0], ei[1]
    owner = dst // NPC

    grow = np.empty(N, np.int64)      # node -> global HB-table row
    percore = []
    for c in range(NCORES):
        m = owner == c
        d_local = dst[m] - c * NPC
        s_c = src[m]
        e_c = ea[m]
        deg = np.bincount(d_local, minlength=NPC)
        order = np.argsort(-deg, kind="stable")   # rank -> local node
        rk = np.empty(NPC, np.int64)
        rk[order] = np.arange(NPC)                # local node -> rank
        degs = deg[order]                         # descending
        grow[c * NPC:(c + 1) * NPC] = c * NPC + (rk % P) * S + rk // P
        percore.append((d_local, s_c, e_c, order, rk, degs))

    Rmax = int(max(pc[5][0] for pc in percore))
    k_r = np.zeros(Rmax, np.int64)
    for pc in percore:
        degs = pc[5]
        a = -degs  # ascending
        cr = np.searchsorted(a, -np.arange(Rmax), side="left")  # count(degs > r)
        k_r = np.maximum(k_r, (cr + P - 1) // P)
    k_r[0] = S
    off = np.zeros(Rmax + 1, np.int64)
    off[1:] = np.cumsum(k_r)
    K_tot = int(off[-1])
    K4 = -(-K_tot // GCOLS) * GCOLS           # pad to a whole number of gathers
    NG = K4 // GCOLS

    # DVE chunks: round 0 split by CHUNK_MAX, then rounds >= 1 split into
    # pieces <= CHUNK_MAX and packed into chunks. rlist entries are
    # (chunk-local offset, agg column offset, length).
    pieces = []  # (abs col, agg col, len)
    for r in range(1, Rmax):
        kr = int(k_r[r])
        a = 0
        while a < kr:
            w = min(CHUNK_MAX, kr - a)
            pieces.append((int(off[r]) + a, a, w))
            a += w
    groups = []  # [col_offset, width, [(local_off, agg_off, len), ...]]
    for (c, a, w) in pieces:
        if groups and groups[-1][1] + w <= CHUNK_MAX:
            g = groups[-1]
            g[2].append((g[1], a, w))
            g[1] += w
        else:
            groups.append([c, w, [(0, a, w)]])
    chunks = [(c0, min(CHUNK_MAX, S - c0), [], True)
              for c0 in range(0, S, CHUNK_MAX)]
    chunks += [(g[0], g[1], g[2], False) for g in groups]

    per_core_arrays = []
    for c, (d_local, s_c, e_c, order, rk, degs) in enumerate(percore):
        r_e = rk[d_local]
        o = np.argsort(r_e, kind="stable")
        rs = r_e[o]
        ss = s_c[o]
        es = e_c[o]
        startpos = np.searchsorted(rs, np.arange(NPC), side="left")
        j = np.arange(len(rs)) - startpos[rs]     # occurrence index = round
        col = off[j] + rs // P
        p = rs % P
        inv_full = np.where(degs > 0, 1.0 / np.maximum(degs, 1), 0.0).astype(
            np.float32)                           # rank -> 1/deg
        sidx = np.zeros((P, K4), np.int64)        # global table row of src
        ea4 = np.zeros((P, K4, 4), np.float32)
        sidx[p, col] = grow[ss]
        ea4[p, col, grow[ss] % 4] = es * inv_full[rs]   # mean-fold: ea/deg(dst)
        sup = (sidx // 4).astype(np.int16)        # super-row (4 nodes / 256 B)

        # dma_gather index layout: gather i covers columns [Gi, Gi+GCOLS);
        # slot j = (col-Gi)*P + p; idx wrapped into 16 partitions, x8 replicas.
        blocks = []
        for i in range(NG):
            flat = sup[:, i * GCOLS:(i + 1) * GCOLS].T.reshape(GIDX)
            blocks.append(flat.reshape(GIDX // 16, 16).T)
        idxg = np.concatenate(blocks, axis=1)     # [16, NG*64]
        idxg = np.tile(idxg, (8, 1)).copy()       # [128, NG*64]

        h0_sel = h0[c * NPC + order]              # [8192, 32] in rank order
        h0_a = np.ascontiguousarray(
            h0_sel.reshape(S, P, DIM).transpose(1, 0, 2)
        )                                          # [128, 64, 32]; h0_a[p,s] = rank s*P+p
        ea4 = ea4.astype(ml_dtypes.bfloat16)
        per_core_arrays.append(dict(h0=h0_a, idxg=idxg, ea4=ea4,
                                    broot=broot))

    bias_nonzero = bool(np.any(conv_bias != 0.0))
    if bias_nonzero:
        bias_a = np.ascontiguousarray(
            np.broadcast_to(conv_bias.reshape(1, 1, DIM), (P, 1, DIM)),
            dtype=np.float32)
        for d in per_core_arrays:
            d["cbias"] = bias_a

    orders = [pc[3] for pc in percore]
    meta = dict(K_tot=K_tot, K4=K4, NG=NG, chunks=chunks,
                bias_nonzero=bias_nonzero, orders=orders,
                lin2_w=np.asarray(inputs["lin2_w"], np.float32),
                lin2_b=np.asarray(inputs["lin2_b"], np.float32))
    return per_core_arrays, meta


def _build_program(K_tot, K4, NG, chunks, bias_nonzero, iters=ITERS):
    from concourse import bacc, bass, mybir, tile
    from concourse import library_config
    from concourse.masks import make_identity

    f32 = mybir.dt.float32
    bf16 = mybir.dt.bfloat16
    i16 = mybir.dt.int16
    MULT = mybir.AluOpType.mult

    nc = bacc.Bacc("TRN2", target_bir_lowering=False, debug=False,
                   num_devices=NCORES, num_swdge_queues=NQUEUES)

    h0_p = nc.dram_tensor("h0", [P, S, DIM], f32, kind="ExternalInput").ap()
    idxg_p = nc.dram_tensor("idxg", [P, NG * (GIDX // 16)], i16,
                            kind="ExternalInput").ap()
    ea4_p = nc.dram_tensor("ea4", [P, K4, 4], bf16, kind="ExternalInput").ap()
    br_p = nc.dram_tensor("broot", [DIM, 2 * DIM], bf16, kind="ExternalInput").ap()
    if bias_nonzero:
        cb_p = nc.dram_tensor("cbias", [P, 1, DIM], f32, kind="ExternalInput").ap()
    hout_p = nc.dram_tensor("h_out", [P, S, DIM], f32, kind="ExternalOutput").ap()

    with tile.TileContext(nc) as tc:
        with (
            tc.tile_pool(name="persist", bufs=1) as pp,
            tc.tile_pool(name="work", bufs=2) as wp,
            tc.tile_pool(name="gpool", bufs=1) as gp,
            tc.tile_pool(name="prodp", bufs=2) as prp,
            tc.tile_pool(name="dramp", bufs=2, space="DRAM") as dp,
            tc.tile_pool(name="pst", bufs=2, space="PSUM") as pst,
            tc.tile_pool(name="psm", bufs=2, space="PSUM") as psm,
        ):
            nc.gpsimd.load_library(library_config.mlp)
            ident = pp.tile([P, P], f32)
            make_identity(nc, ident[:])
            h = pp.tile([P, S, DIM], f32)
            idxg_sb = pp.tile([P, NG * (GIDX // 16)], i16)
            ea4_sb = pp.tile([P, K4, 4], bf16)
            br_sb = pp.tile([DIM, 2 * DIM], bf16)
            gbuf = pp.tile([P, K4, 4 * DIM], bf16)
            nc.sync.dma_start(out=h[:], in_=h0_p[:])
            nc.sync.dma_start(out=idxg_sb[:], in_=idxg_p[:])
            nc.sync.dma_start(out=ea4_sb[:], in_=ea4_p[:])
            nc.sync.dma_start(out=br_sb[:], in_=br_p[:])
            if bias_nonzero:
                cb_sb = pp.tile([P, 1, DIM], f32)
                nc.sync.dma_start(out=cb_sb[:], in_=cb_p[:])

            for it in range(iters):
                # hT[:, c, :] = h[:, c, :]^T  (feature-major copy for matmul lhsT)
                hT = wp.tile([DIM, S, P], bf16)
                for tb in range(S // 4):
                    pt = pst.tile([DIM, 4, P], f32)
                    for b in range(4):
                        nc.tensor.transpose(out=pt[:, b, :], in_=h[:, tb * 4 + b, :],
                                            identity=ident[:])
                    nc.any.tensor_copy(out=hT[:, tb * 4:tb * 4 + 4, :], in_=pt[:])

                # [HB | hR] = h @ [B | root], node-major
                # HB is AllGathered + gathered in bf16 to halve DMA bytes
                hbc = wp.tile([P, S, DIM], bf16)
                hr = wp.tile([P, S, DIM], bf16)
                for mb in range(S // 8):
                    pm = psm.tile([P, 8, 2 * DIM], f32)
                    for b in range(8):
                        cidx = mb * 8 + b
                        nc.tensor.matmul(out=pm[:, b, :], lhsT=hT[:, cidx, :],
                                         rhs=br_sb[:], start=True, stop=True)
                    nc.any.tensor_copy(out=hbc[:, mb * 8:mb * 8 + 8, :],
                                       in_=pm[:, :, 0:DIM])
                    nc.any.tensor_copy(out=hr[:, mb * 8:mb * 8 + 8, :],
                                       in_=pm[:, :, DIM:2 * DIM])

                bounce = dp.tile([NPC, DIM], bf16)
                hbf = dp.tile([N_NODES, DIM], bf16, addr_space="Shared")
                nc.sync.dma_start(
                    out=bounce[:].rearrange("(p s) d -> p s d", p=P), in_=hbc[:])
                nc.gpsimd.collective_compute(
                    "AllGather", mybir.AluOpType.bypass,
                    replica_groups=[list(range(NCORES))],
                    ins=[bounce.opt()], outs=[hbf.opt()],
                )
                hbf4 = hbf[:].rearrange("(s f) d -> s (f d)", f=4)  # [16384, 128]

                # batched gathers of 256-B super-rows (4 nodes each).
                # queue = (sem % 8) % NQUEUES keeps the tile DMASW sem
                # rotation (mod 8) consistently paired with SWDGE queues.
                for i in range(NG):
                    g_ctr = it * NG + i
                    nc.gpsimd.dma_gather(
                        gbuf[:, i * GCOLS:(i + 1) * GCOLS, :], hbf4,
                        idxg_sb[:, i * (GIDX // 16):(i + 1) * (GIDX // 16)],
                        GIDX, GIDX, 4 * DIM,
                        queue_num=(g_ctr % 8) % NQUEUES)

                # mask-expand ea on Act, 2x-mode bf16 multiply + fold on DVE,
                # then segment-sum (ELL rounds)
                agg = wp.tile([P, S, DIM], bf16)
                for (coff, width, rlist, is_first) in chunks:
                    eax = prp.tile([P, CHUNK_MAX, 4 * DIM], bf16, name="eax")
                    nc.scalar.activation(
                        out=eax[:, 0:width, :].rearrange(
                            "p w (m f) -> p w m f", m=4),
                        in_=ea4_sb[:, coff:coff + width, :].unsqueeze(3)
                            .to_broadcast([P, width, 4, DIM]),
                        func=mybir.ActivationFunctionType.Copy)
                    t128 = prp.tile([P, CHUNK_MAX, 4 * DIM], bf16, name="t128")
                    nc.vector.tensor_tensor(
                        out=t128[:, 0:width, :],
                        in0=gbuf[:, coff:coff + width, :],
                        in1=eax[:, 0:width, :], op=MULT)
                    ta = prp.tile([P, CHUNK_MAX, DIM], bf16, name="ta")
                    tb = prp.tile([P, CHUNK_MAX, DIM], bf16, name="tb")
                    nc.vector.tensor_add(out=ta[:, 0:width, :],
                                         in0=t128[:, 0:width, 0:DIM],
                                         in1=t128[:, 0:width, DIM:2 * DIM])
                    nc.vector.tensor_add(out=tb[:, 0:width, :],
                                         in0=t128[:, 0:width, 2 * DIM:3 * DIM],
                                         in1=t128[:, 0:width, 3 * DIM:4 * DIM])
                    if is_first:
                        nc.vector.tensor_add(out=agg[:, coff:coff + width, :],
                                             in0=ta[:, 0:width, :],
                                             in1=tb[:, 0:width, :])
                    else:
                        nc.vector.tensor_add(out=ta[:, 0:width, :],
                                             in0=ta[:, 0:width, :],
                                             in1=tb[:, 0:width, :])
                        for (lo, aoff, kr) in rlist:
                            nc.vector.tensor_add(
                                out=agg[:, aoff:aoff + kr, :],
                                in0=agg[:, aoff:aoff + kr, :],
                                in1=ta[:, lo:lo + kr, :])

                # h += relu(agg + h @ root (+ bias)); 1/deg folded into ea4
                nc.vector.tensor_add(out=agg[:], in0=agg[:], in1=hr[:])
                if bias_nonzero:
                    nc.vector.tensor_add(out=agg[:], in0=agg[:],
                                         in1=cb_sb[:].to_broadcast([P, S, DIM]))
                nc.scalar.activation(out=agg[:], in_=agg[:],
                                     func=mybir.ActivationFunctionType.Relu)
                nc.vector.tensor_add(out=h[:], in0=h[:], in1=agg[:])

            nc.sync.dma_start(out=hout_p[:], in_=h[:])

    nc.compile()
    return nc


TIME_K = 9        # chained executions in the timing jit
TIME_REPS = 5     # wall-clock repetitions, take min
_RUNNERS = {}


def _pjrt_runner(nc):
    import jax
    from jax.experimental.shard_map import shard_map
    from jax.sharding import Mesh, NamedSharding, PartitionSpec
    from concourse import mybir
    from concourse.bass2jax import (_bass_exec_p, install_neuronx_cc_hook,
                                    partition_id_tensor)

    install_neuronx_cc_hook()

    partition_name = nc.partition_id_tensor.name if nc.partition_id_tensor else None
    in_names, out_names, out_avals = [], [], []
    for alloc in nc.m.functions[0].allocations:
        if not isinstance(alloc, mybir.MemoryLocationSet):
            continue
        name = alloc.memorylocations[0].name
        if alloc.kind == "ExternalInput":
            if name != partition_name:
                in_names.append(name)
        elif alloc.kind == "ExternalOutput":
            out_names.append(name)
            out_avals.append(jax.core.ShapedArray(
                tuple(alloc.tensor_shape), mybir.dt.np(alloc.dtype)))
    n_params = len(in_names)
    all_names = tuple(in_names) + tuple(out_names) + (
        (partition_name,) if partition_name else ())

    def bind(ins, carries):
        ops = list(ins) + list(carries)
        if partition_name is not None:
            ops.append(partition_id_tensor())
        return _bass_exec_p.bind(
            *ops, out_avals=tuple(out_avals), in_names=all_names,
            out_names=tuple(out_names), lowering_input_output_aliases=(),
            sim_require_finite=True, sim_require_nnan=True, nc=nc)

    def body1(*args):
        return tuple(bind(args[:n_params], args[n_params:]))

    devices = jax.devices()[:NCORES]
    mesh = Mesh(np.asarray(devices), ("core",))
    spec = PartitionSpec("core")
    nio = n_params + len(out_names)
    f1 = jax.jit(shard_map(body1, mesh=mesh, in_specs=(spec,) * nio,
                           out_specs=(spec,) * len(out_names), check_rep=False))
    sharding = NamedSharding(mesh, spec)
    return dict(in_names=in_names, out_names=out_names, out_avals=out_avals,
                sharding=sharding, f1=f1, jax=jax)


def _pjrt_run_maps(nc, in_maps, time_it=False):
    global LAST_EXEC_NS, LAST_PROFILE
    import time as _time
    r = _RUNNERS.get(id(nc))
    if r is None:
        r = _pjrt_runner(nc)
        _RUNNERS[id(nc)] = r
    jax = r["jax"]
    concat_in = [np.concatenate([in_maps[c][nm] for c in range(NCORES)], axis=0)
                 for nm in r["in_names"]]
    zeros = [np.zeros((NCORES * a.shape[0], *a.shape[1:]), a.dtype)
             for a in r["out_avals"]]
    dev_in = [jax.device_put(x, r["sharding"]) for x in concat_in]
    dev_zero = [jax.device_put(z, r["sharding"]) for z in zeros]

    outs = jax.block_until_ready(r["f1"](*dev_in, *dev_zero))

    if time_it:
        # One bass_exec per jit module is allowed, so chain K executions by
        # issuing K async dispatches back-to-back; they queue on-device and
        # the slope vs a single blocked call removes the host/tunnel RTT.
        t1 = tk = float("inf")
        for _ in range(TIME_REPS):
            t0 = _time.perf_counter()
            jax.block_until_ready(r["f1"](*dev_in, *dev_zero))
            t1 = min(t1, _time.perf_counter() - t0)
            t0 = _time.perf_counter()
            rs = [r["f1"](*dev_in, *dev_zero) for _ in range(TIME_K)]
            jax.block_until_ready(rs)
            tk = min(tk, _time.perf_counter() - t0)
        LAST_EXEC_NS = int((tk - t1) / (TIME_K - 1) * 1e9)
        LAST_PROFILE = {"t1_ns": int(t1 * 1e9), "tK_ns": int(tk * 1e9),
                        "K": TIME_K}

    out_full = np.asarray(outs[0]).reshape(NCORES, *r["out_avals"][0].shape)
    return [out_full[c] for c in range(NCORES)]


def _run(nc, per_core_arrays):
    in_maps = [dict(d) for d in per_core_arrays]

    if USE_SIM:
        from concourse.bass_interp import MultiCoreSim
        sim = MultiCoreSim(nc, num_cores=NCORES)
        for i in range(NCORES):
            for k, v in in_maps[i].items():
                sim.cores[i].tensor(k)[:] = v
        sim.simulate()
        return [np.array(sim.cores[i].tensor("h_out")) for i in range(NCORES)]

    return _pjrt_run_maps(nc, in_maps, time_it=TRACE)


def kernel(**inputs):
    per_core_arrays, meta = _preprocess(inputs)

    key = (meta["K_tot"], meta["K4"],
           tuple((c[0], c[1]) for c in meta["chunks"]),
           meta["bias_nonzero"])
    nc = _CACHE.get(key)
    if nc is None:
        nc = _build_program(meta["K_tot"], meta["K4"], meta["NG"],
                            meta["chunks"], meta["bias_nonzero"])
        _CACHE[key] = nc

    outs = _run(nc, per_core_arrays)

    h_full = np.empty((N_NODES, DIM), np.float32)
    for c in range(NCORES):
        by_rank = np.asarray(outs[c]).reshape(P, S, DIM).transpose(1, 0, 2).reshape(NPC, DIM)
        h_full[c * NPC + meta["orders"][c]] = by_rank
    g = np.mean(h_full, axis=0, dtype=np.float64).astype(np.float32)
    out = (g @ meta["lin2_w"] + meta["lin2_b"]).reshape(-1).astype(np.float32)
    return out
